# revision 46
# baseline (speedup 1.0000x reference)
"""Causal single-head attention on 8 Trainium2 NeuronCores — fully local.

Problem: x[4096,1024] -> Q,K,V = x@W.T+b (d_k=64), out = softmax(causal(QK^T/8)) @ V.

Strategy (replicated K/V, packed stream, zero communication):
  - Every core computes K^T and V~ for all 32 key blocks locally; no
    collective, no cross-core sync of any kind.
  - Core c owns query blocks {c, 8+c, 16+c, 24+c} (strided) -> every core
    runs the IDENTICAL program. Its 512 own x columns ride in a dedicated
    1MB xqT input (used for BOTH the Q projection and the own-block K/V
    projection); the main x stream is host-PACKED per core with those
    columns deleted (7MB instead of 8MB).
  - Packed coordinates stay rank-uniform: slot j attends exactly 7j+7
    packed key blocks; the causal diagonal band sits at packed positions
    7j..7j+6 where a per-core all-0/all-1 block mask (with 3 leading
    ones-blocks padding group windows) selects earlier-rank blocks. The
    triangular mask lives in a STATIC own-vs-own path (own key blocks
    i <= j, tri exactly when i == j — rank-independent).
  - x streams in 512-col chunks (last 512 as two 256-col sub-chunks to
    shorten the tail chain); each chunk's K^T/V^T projection runs as TWO
    interleaved 256-col PSUM chains in ONE open accumulation group; V~
    blocks are PE transposes of V^T with a ones-column appended so the AV
    matmul also accumulates the softmax denominator.
  - Attention (scores -> exp -> mask -> AV) for a (slot, key-group) pair is
    emitted as soon as the group's blocks finish projecting; each slot's
    epilogue (transpose, 1/rowsum, store) fires as soon as its accumulation
    completes, overlapping later chunks.
  - All attention matmuls in bf16 (rate-1 at any moving width); exp on
    ScalarE with the 1/8 scale folded in; accumulation in f32 PSUM.
  - Constants ride in packed blobs to amortize per-DMA overhead.

  PSUM discipline (hardware-verified): a PSUM bank supports ONE open
  accumulation group at a time. Interleaving two start/stop chains in one
  bank silently corrupts accumulation on hardware (the simulator does not
  model this). Hence: the kv projection's two interleaved column chains
  share a single group (start on the very first matmul, stop on the last,
  per-element has_written handles overwrite-vs-accumulate); each (chunk,
  slot) AV block runs as a CLOSED group (start..stop) in a rotating scratch
  bank and is accumulated into per-slot SBUF tiles on VectorE.
"""

import os
import numpy as np
import ml_dtypes
from contextlib import ExitStack

S, DM, DK = 4096, 1024, 64
NCORES = 8
QB = 128                      # rows per block
SLOTS = 4                     # q-blocks per core
SH = QB * SLOTS               # 512 own query rows per core
NB = S // QB                  # 32 key blocks
SP = S - SH                   # 3584 packed (non-own) x columns
NPB = SP // QB                # 28 packed key blocks
CHUNK = 512                   # x columns per streamed chunk
ND = DM // 128                # 8 contraction chunks

# cb_w bf16 blob (early): wkv [8, 128] | ident [128]
BF_WKV = 0
BF_ID = ND * 128
BFW_COLS = BF_ID + 128
# cb_r bf16 blob (later): wq [8, 64] | band mask [10, 128] | tri [128]
BFR_WQ = 0
BFR_MASK = ND * DK
BFR_TRI = BFR_MASK + 10 * QB
BFR_COLS = BFR_TRI + QB
# f32 blob layout: bkv [1] | bq [1] | identf [128]
F_COLS = 2 + 128

AMP = int(os.environ.get("KERNEL_AMP", "1"))  # repeat whole pipeline in-NEFF

LAST_EXEC_NS = None


def _build_nc():
    import concourse.bass as bass
    import concourse.bacc as bacc
    import concourse.mybir as mybir
    import concourse.tile as tile

    f32 = mybir.dt.float32
    bf16 = mybir.dt.bfloat16
    AF = mybir.ActivationFunctionType

    nc = bacc.Bacc(None, num_devices=NCORES)

    xT_d = nc.dram_tensor("xT", [DM, SP], bf16, kind="ExternalInput")
    xqT_d = nc.dram_tensor("xqT", [DM, SH], bf16, kind="ExternalInput")
    cbw_d = nc.dram_tensor("cbw", [128, BFW_COLS], bf16, kind="ExternalInput")
    cbr_d = nc.dram_tensor("cbr", [128, BFR_COLS], bf16, kind="ExternalInput")
    cf_d = nc.dram_tensor("cf", [128, F_COLS], f32, kind="ExternalInput")
    out_d = nc.dram_tensor("out", [SH, DK], f32, kind="ExternalOutput")

    with tile.TileContext(nc) as tc, ExitStack() as ctx:
        singles = ctx.enter_context(tc.tile_pool(name="singles", bufs=1))
        psA = ctx.enter_context(tc.tile_pool(name="psA", bufs=1, space="PSUM"))
        psB = ctx.enter_context(tc.tile_pool(name="psB", bufs=2, space="PSUM"))
        epool = ctx.enter_context(tc.tile_pool(name="epool", bufs=4))

        # ---------------- packed constant loads ----------------
        cbw_sb = singles.tile([128, BFW_COLS], bf16)
        nc.sync.dma_start(out=cbw_sb, in_=cbw_d[:, :])
        cf_sb = singles.tile([128, F_COLS], f32)
        nc.sync.dma_start(out=cf_sb, in_=cf_d[:, :])
        cbr_sb = singles.tile([128, BFR_COLS], bf16)

        wkv_sb = cbw_sb[:, BF_WKV:BF_ID].rearrange("p (d c) -> p d c", d=ND)
        ident_sb = cbw_sb[:, BF_ID:BF_ID + 128]
        wq_sb = cbr_sb[:, BFR_WQ:BFR_MASK].rearrange("p (d c) -> p d c", d=ND)
        mask_sb = cbr_sb[:, BFR_MASK:BFR_TRI].rearrange(
            "p (kb q) -> p kb q", kb=10)
        tri_sb = cbr_sb[:, BFR_TRI:BFR_COLS]
        bkv_sb = cf_sb[:, 0:1]
        bq_sb = cf_sb[0:DK, 1:2]
        identf_sb = cf_sb[:, 2:2 + 128]

        xT_sb = singles.tile([128, ND, SP], bf16)
        xq_sb = singles.tile([128, ND, SH], bf16)
        kT_sb = singles.tile([DK, SP + SH], bf16)   # packed | own
        vt_sb = singles.tile([128, NB, DK + 1], bf16)  # packed | own
        qT_sb = singles.tile([DK, SH], bf16)
        # ones column of V~ (denominator accumulator)
        nc.scalar.activation(vt_sb[:, :, DK:DK + 1], ident_sb[:, 0:NB],
                             AF.Identity, bias=1.0, scale=0.0)

        def load_cols(c0, c1):
            cs = slice(c0, c1)
            nc.sync.dma_start(
                out=xT_sb[:, :, cs],
                in_=xT_d[:, cs].rearrange("(d p) s -> p d s", p=128))

        def one_pass(rep):
            load_cols(0, 256)
            load_cols(256, 512)
            nc.sync.dma_start(out=xq_sb, in_=xqT_d[:, :].rearrange(
                "(d p) s -> p d s", p=128))
            nc.sync.dma_start(out=cbr_sb, in_=cbr_d[:, :])
            for c0 in range(CHUNK, SP - 512, CHUNK):
                load_cols(c0, c0 + CHUNK)
            load_cols(SP - 512, SP - 256)
            load_cols(SP - 256, SP)

            av_started = [False] * SLOTS
            av_acc = [singles.tile([DK + 1, QB], f32, name=f"av_acc{j}")
                      for j in range(SLOTS)]

            def emit_proj(c0, W=CHUNK, xsrc=None, kbase=0):
                # project x cols [c0, c0+W) (from the packed stream by
                # default, or from xq for the own blocks) into kT/vt at
                # block base kbase + c0//QB
                xs = xsrc if xsrc is not None else xT_sb
                h = W // 2
                cs0 = slice(c0, c0 + h)
                cs1 = slice(c0 + h, c0 + W)
                kv_ps = psB.tile([128, 2, 256], f32, tag="kvps", bufs=2,
                                 name="kv_ps")
                kv0, kv1 = kv_ps[:, 0, 0:h], kv_ps[:, 1, 0:h]
                for d in range(ND):
                    nc.tensor.matmul(kv0, lhsT=wkv_sb[:, d, :],
                                     rhs=xs[:, d, cs0],
                                     start=(d == 0), stop=False,
                                     skip_group_check=True)
                    nc.tensor.matmul(kv1, lhsT=wkv_sb[:, d, :],
                                     rhs=xs[:, d, cs1],
                                     start=False, stop=(d == ND - 1),
                                     skip_group_check=True)
                kc0 = kbase * QB + c0
                nc.vector.tensor_scalar_add(
                    kT_sb[:, kc0:kc0 + W].rearrange("k (h s) -> k h s", h=2),
                    kv_ps[0:DK, :, 0:h], bkv_sb[0:DK, 0:1])
                vT_h = epool.tile([DK, CHUNK], f32, tag="vth", name="vT_h")
                nc.vector.tensor_scalar_add(
                    vT_h[:, 0:W].rearrange("k (h s) -> k h s", h=2),
                    kv_ps[DK:128, :, 0:h], bkv_sb[DK:128, 0:1])
                t_ps = psB.tile([128, 4, QB], f32, tag="scps", bufs=3,
                                name="t_ps")
                for sl in range(W // QB):
                    nc.tensor.transpose(t_ps[:, sl, 0:DK],
                                        vT_h[:, QB * sl:QB * (sl + 1)],
                                        identf_sb[0:DK, 0:DK])
                kb0 = kbase + c0 // QB
                nc.scalar.copy(vt_sb[:, kb0:kb0 + W // QB, 0:DK],
                               t_ps[:, 0:W // QB, 0:DK])

            def emit_attn(kb0, nb, j):
                # attention for q-slot j vs key blocks [kb0, kb0+nb) of the
                # PACKED stream (slot j's packed prefix is 7j+7 blocks;
                # band = packed positions 7j..7j+6, 0/1 mask by rank)
                qc = slice(QB * j, QB * (j + 1))
                sc_ps = psB.tile([128, 4, QB], f32, tag="scps",
                                 bufs=3, name="sc_ps")
                e_sb = epool.tile([128, 4, QB], bf16, tag=f"e{j % 2}",
                                  name="e_sb")
                for sl in range(nb):
                    kb = kb0 + sl
                    nc.tensor.matmul(sc_ps[:, sl, :],
                                     lhsT=kT_sb[:, QB * kb:QB * (kb + 1)],
                                     rhs=qT_sb[:, qc],
                                     start=True, stop=True)
                nc.scalar.activation(e_sb[:, 0:nb, :], sc_ps[:, 0:nb, :],
                                     AF.Exp, scale=0.125)
                band0 = 7 * j
                if kb0 + nb > band0:   # overlaps the band
                    mi = 3 + kb0 - band0   # mask has 3 leading ones blocks
                    nc.vector.tensor_mul(e_sb[:, 0:nb, :], e_sb[:, 0:nb, :],
                                         mask_sb[:, mi:mi + nb, :])
                avp = psB.tile([DK + 1, QB], f32, tag="avp", bufs=2,
                               name="avp")
                for sl in range(nb):
                    kb = kb0 + sl
                    nc.tensor.matmul(avp, lhsT=vt_sb[:, kb, :],
                                     rhs=e_sb[:, sl, :],
                                     start=(sl == 0), stop=(sl == nb - 1),
                                     skip_group_check=True)
                if not av_started[j]:
                    nc.vector.tensor_copy(av_acc[j], avp)
                    av_started[j] = True
                else:
                    nc.vector.tensor_tensor(av_acc[j], av_acc[j], avp,
                                            op=mybir.AluOpType.add)

            def emit_own(j):
                # own key blocks i <= j (tri mask on i == j, static)
                nbo = j + 1
                sc_ps = psB.tile([128, 4, QB], f32, tag="scps",
                                 bufs=3, name="sc_ps")
                e_sb = epool.tile([128, 4, QB], bf16, tag=f"e{j % 2}",
                                  name="e_sb")
                for i in range(nbo):
                    nc.tensor.matmul(
                        sc_ps[:, i, :],
                        lhsT=kT_sb[:, SP + QB * i:SP + QB * (i + 1)],
                        rhs=qT_sb[:, QB * j:QB * (j + 1)],
                        start=True, stop=True)
                nc.scalar.activation(e_sb[:, 0:nbo, :], sc_ps[:, 0:nbo, :],
                                     AF.Exp, scale=0.125)
                nc.vector.tensor_mul(e_sb[:, j, :], e_sb[:, j, :], tri_sb)
                avp = psB.tile([DK + 1, QB], f32, tag="avp", bufs=2,
                               name="avp")
                for i in range(nbo):
                    nc.tensor.matmul(avp, lhsT=vt_sb[:, NPB + i, :],
                                     rhs=e_sb[:, i, :],
                                     start=(i == 0), stop=(i == nbo - 1),
                                     skip_group_check=True)
                if not av_started[j]:
                    nc.vector.tensor_copy(av_acc[j], avp)
                    av_started[j] = True
                else:
                    nc.vector.tensor_tensor(av_acc[j], av_acc[j], avp,
                                            op=mybir.AluOpType.add)

            def emit_epi(j):
                t2 = psB.tile([128, 4, QB], f32, tag="scps", bufs=3,
                              name="t2")
                nc.tensor.transpose(t2[:, 0, 0:DK + 1], av_acc[j],
                                    identf_sb[0:DK + 1, 0:DK + 1])
                rec = epool.tile([128, 1], f32, tag="rec", name="rec")
                nc.vector.reciprocal(rec, t2[:, 0, DK:DK + 1])
                out_sb = epool.tile([128, DK], f32, tag="osb",
                                    name="out_sb")
                nc.vector.tensor_scalar_mul(out_sb, t2[:, 0, 0:DK], rec)
                nc.sync.dma_start(out=out_d[QB * j:QB * (j + 1), :],
                                  in_=out_sb)

            def attn_for_ready(lo, hi):
                # emit attention for every (slot, group) whose packed blocks
                # lie in [lo, hi) -- groups are per-slot, 4-block aligned to
                # the slot's own prefix
                for j in range(SLOTS):
                    pref = 7 * j + 7
                    for g0 in range(0, pref, 4):
                        nb = min(4, pref - g0)
                        if lo < g0 + nb <= hi:
                            emit_attn(g0, nb, j)
                            if g0 + nb == pref:
                                emit_epi(j)

            emit_proj(0, CHUNK)
            # Q^T and own-block K/V from xq
            q_ps = psA.tile([DK, SH], f32, name="q_ps", tag="qps")
            for d in range(ND):
                nc.tensor.matmul(q_ps, lhsT=wq_sb[:, d, :],
                                 rhs=xq_sb[:, d, :],
                                 start=(d == 0), stop=(d == ND - 1))
            nc.scalar.activation(qT_sb, q_ps, AF.Identity,
                                 bias=bq_sb[:, 0:1], scale=1.0)
            emit_proj(0, CHUNK, xsrc=xq_sb, kbase=NPB)
            for j in range(SLOTS):
                emit_own(j)
            done = 4
            attn_for_ready(0, 4)
            for c0 in range(CHUNK, SP - 512, CHUNK):
                emit_proj(c0)
                attn_for_ready(done, done + 4)
                done += 4
            emit_proj(SP - 512, 256)
            attn_for_ready(done, done + 2)
            done += 2
            emit_proj(SP - 256, 256)
            attn_for_ready(done, done + 2)

        for _rep in range(AMP):
            one_pass(_rep)

    nc.finalize()
    return nc


def _in_maps(x, Wq, bq, Wk, bk, Wv, bv):
    bf = ml_dtypes.bfloat16
    xT = np.ascontiguousarray(x.T).astype(bf)                      # [1024, 4096]
    tri = np.triu(np.ones((QB, QB), dtype=np.float32))  # E^T[k,q] valid iff k<=q

    # bf16 constant blobs
    wkvT = np.concatenate([Wk.T, Wv.T], axis=1)                    # [1024, 128]
    wkv_p = wkvT.reshape(ND, 128, 2 * DK).transpose(1, 0, 2).reshape(128, -1)
    wqT = Wq.T                                                     # [1024, 64]
    wq_p = wqT.reshape(ND, 128, DK).transpose(1, 0, 2).reshape(128, -1)
    ident = np.eye(128, dtype=np.float32)
    cbw = np.ascontiguousarray(
        np.concatenate([wkv_p, ident], axis=1).astype(bf))
    assert cbw.shape == (128, BFW_COLS)

    # f32 constants
    cf = np.zeros((128, F_COLS), dtype=np.float32)
    cf[:, 0] = np.concatenate([bk, bv])
    cf[0:DK, 1] = bq
    cf[:, 2:2 + 128] = ident

    maps = []
    for c in range(NCORES):
        own = [8 * sl + c for sl in range(SLOTS)]
        rows = np.concatenate([np.arange(QB * b, QB * (b + 1)) for b in own])
        xqT = np.ascontiguousarray(x[rows].T).astype(bf)           # [1024, 512]
        own_cols = rows
        keep = np.setdiff1d(np.arange(S), own_cols)
        xTp = np.ascontiguousarray(xT[:, keep])                    # [1024, 3584]
        # band mask (packed positions of blocks 8j..8j+7 minus own, same for
        # every j): position p_rel < c -> earlier rank block (valid), else 0.
        # 3 leading all-ones pad blocks absorb group windows that start
        # before the band.
        m = np.zeros((10, QB, QB), dtype=np.float32)
        m[0:3] = 1.0
        for p_rel in range(7):
            if p_rel < c:
                m[3 + p_rel] = 1.0
        mask_p = m.transpose(1, 0, 2).reshape(128, -1)
        cbr = np.ascontiguousarray(np.concatenate(
            [wq_p, mask_p, tri], axis=1).astype(bf))
        assert cbr.shape == (128, BFR_COLS)
        maps.append({"xT": xTp, "xqT": xqT, "cbw": cbw, "cbr": cbr,
                     "cf": cf})
    return maps


def kernel(**inputs):
    global LAST_EXEC_NS
    x = np.asarray(inputs["x"], dtype=np.float32)
    args = [np.asarray(inputs[k], dtype=np.float32)
            for k in ("Wq", "bq", "Wk", "bk", "Wv", "bv")]
    in_maps = _in_maps(x, args[0], args[1], args[2], args[3], args[4], args[5])

    nc = _build_nc()
    from concourse.bass_utils import run_bass_kernel_spmd
    res = run_bass_kernel_spmd(nc, in_maps, core_ids=list(range(NCORES)))
    LAST_EXEC_NS = res.exec_time_ns

    out = np.zeros((S, DK), dtype=np.float32)
    for c in range(NCORES):
        r = res.results[c]["out"]
        for sl in range(SLOTS):
            b = 8 * sl + c
            out[QB * b:QB * (b + 1)] = r[QB * sl:QB * (sl + 1)]
    return out


# revision 47
# speedup vs baseline: 1.0177x; 1.0177x over previous
"""Causal single-head attention on 8 Trainium2 NeuronCores — fully local.

Problem: x[4096,1024] -> Q,K,V = x@W.T+b (d_k=64), out = softmax(causal(QK^T/8)) @ V.

Strategy (replicated K/V, packed stream, zero communication):
  - Every core computes K^T and V~ for all 32 key blocks locally; no
    collective, no cross-core sync of any kind.
  - Core c owns query blocks {c, 8+c, 16+c, 24+c} (strided) -> every core
    runs the IDENTICAL program. Its 512 own x columns ride in a dedicated
    1MB xqT input (used for BOTH the Q projection and the own-block K/V
    projection); the main x stream is host-PACKED per core with those
    columns deleted (7MB instead of 8MB).
  - Packed coordinates stay rank-uniform: slot j attends exactly 7j+7
    packed key blocks; the causal diagonal band sits at packed positions
    7j..7j+6 where a per-core all-0/all-1 block mask (with 3 leading
    ones-blocks padding group windows) selects earlier-rank blocks. The
    triangular mask lives in a STATIC own-vs-own path (own key blocks
    i <= j, tri exactly when i == j — rank-independent).
  - x streams in 512-col chunks (last 512 as two 256-col sub-chunks to
    shorten the tail chain); each chunk's K^T/V^T projection runs as TWO
    interleaved 256-col PSUM chains in ONE open accumulation group; V~
    blocks are PE transposes of V^T with a ones-column appended so the AV
    matmul also accumulates the softmax denominator.
  - Attention (scores -> exp -> mask -> AV) for a (slot, key-group) pair is
    emitted as soon as the group's blocks finish projecting; each slot's
    epilogue (transpose, 1/rowsum, store) fires as soon as its accumulation
    completes, overlapping later chunks.
  - All attention matmuls in bf16 (rate-1 at any moving width); exp on
    ScalarE with the 1/8 scale folded in; accumulation in f32 PSUM.
  - Constants ride in packed blobs to amortize per-DMA overhead.

  PSUM discipline (hardware-verified): a PSUM bank supports ONE open
  accumulation group at a time. Interleaving two start/stop chains in one
  bank silently corrupts accumulation on hardware (the simulator does not
  model this). Hence: the kv projection's two interleaved column chains
  share a single group (start on the very first matmul, stop on the last,
  per-element has_written handles overwrite-vs-accumulate); each (chunk,
  slot) AV block runs as a CLOSED group (start..stop) in a rotating scratch
  bank and is accumulated into per-slot SBUF tiles on VectorE.
"""

import os
import numpy as np
import ml_dtypes
from contextlib import ExitStack

S, DM, DK = 4096, 1024, 64
NCORES = 8
QB = 128                      # rows per block
SLOTS = 4                     # q-blocks per core
SH = QB * SLOTS               # 512 own query rows per core
NB = S // QB                  # 32 key blocks
SP = S - SH                   # 3584 packed (non-own) x columns
NPB = SP // QB                # 28 packed key blocks
CHUNK = 512                   # x columns per streamed chunk
ND = DM // 128                # 8 contraction chunks

# cb_w bf16 blob (early): wkv [8, 128] | wq [8, 64] | ident [128]
BF_WKV = 0
BF_WQ = ND * 128
BF_ID = BF_WQ + ND * DK
BFW_COLS = BF_ID + 128
# cb_r bf16 blob (later): band mask [10, 128] | tri [128]
BFR_MASK = 0
BFR_TRI = BFR_MASK + 10 * QB
BFR_COLS = BFR_TRI + QB
# f32 blob layout: bkv [1] | bq [1] | identf [128]
F_COLS = 2 + 128

AMP = int(os.environ.get("KERNEL_AMP", "1"))  # repeat whole pipeline in-NEFF

LAST_EXEC_NS = None


def _build_nc():
    import concourse.bass as bass
    import concourse.bacc as bacc
    import concourse.mybir as mybir
    import concourse.tile as tile

    f32 = mybir.dt.float32
    bf16 = mybir.dt.bfloat16
    AF = mybir.ActivationFunctionType

    nc = bacc.Bacc(None, num_devices=NCORES)

    xT_d = nc.dram_tensor("xT", [DM, SP], bf16, kind="ExternalInput")
    xqT_d = nc.dram_tensor("xqT", [DM, SH], bf16, kind="ExternalInput")
    cbw_d = nc.dram_tensor("cbw", [128, BFW_COLS], bf16, kind="ExternalInput")
    cbr_d = nc.dram_tensor("cbr", [128, BFR_COLS], bf16, kind="ExternalInput")
    cf_d = nc.dram_tensor("cf", [128, F_COLS], f32, kind="ExternalInput")
    out_d = nc.dram_tensor("out", [SH, DK], f32, kind="ExternalOutput")

    with tile.TileContext(nc) as tc, ExitStack() as ctx:
        singles = ctx.enter_context(tc.tile_pool(name="singles", bufs=1))
        psA = ctx.enter_context(tc.tile_pool(name="psA", bufs=1, space="PSUM"))
        psB = ctx.enter_context(tc.tile_pool(name="psB", bufs=2, space="PSUM"))
        epool = ctx.enter_context(tc.tile_pool(name="epool", bufs=4))

        # ---------------- packed constant loads ----------------
        cbw_sb = singles.tile([128, BFW_COLS], bf16)
        nc.sync.dma_start(out=cbw_sb, in_=cbw_d[:, :])
        cf_sb = singles.tile([128, F_COLS], f32)
        nc.sync.dma_start(out=cf_sb, in_=cf_d[:, :])
        cbr_sb = singles.tile([128, BFR_COLS], bf16)

        wkv_sb = cbw_sb[:, BF_WKV:BF_WQ].rearrange("p (d c) -> p d c", d=ND)
        wq_sb = cbw_sb[:, BF_WQ:BF_ID].rearrange("p (d c) -> p d c", d=ND)
        ident_sb = cbw_sb[:, BF_ID:BF_ID + 128]
        mask_sb = cbr_sb[:, BFR_MASK:BFR_TRI].rearrange(
            "p (kb q) -> p kb q", kb=10)
        tri_sb = cbr_sb[:, BFR_TRI:BFR_COLS]
        bkv_sb = cf_sb[:, 0:1]
        bq_sb = cf_sb[0:DK, 1:2]
        identf_sb = cf_sb[:, 2:2 + 128]

        xT_sb = singles.tile([128, ND, SP], bf16)
        xq_sb = singles.tile([128, ND, SH], bf16)
        kT_sb = singles.tile([DK, SP + SH], bf16)   # packed | own
        vt_sb = singles.tile([128, NB, DK + 1], bf16)  # packed | own
        qT_sb = singles.tile([DK, SH], bf16)
        # ones column of V~ (denominator accumulator)
        nc.scalar.activation(vt_sb[:, :, DK:DK + 1], ident_sb[:, 0:NB],
                             AF.Identity, bias=1.0, scale=0.0)

        def load_cols(c0, c1):
            cs = slice(c0, c1)
            nc.sync.dma_start(
                out=xT_sb[:, :, cs],
                in_=xT_d[:, cs].rearrange("(d p) s -> p d s", p=128))

        def one_pass(rep):
            load_cols(0, 256)
            load_cols(256, 512)
            nc.sync.dma_start(out=xq_sb, in_=xqT_d[:, :].rearrange(
                "(d p) s -> p d s", p=128))
            nc.sync.dma_start(out=cbr_sb, in_=cbr_d[:, :])
            for c0 in range(CHUNK, SP - 512, CHUNK):
                load_cols(c0, c0 + CHUNK)
            load_cols(SP - 512, SP - 256)
            load_cols(SP - 256, SP)

            av_started = [False] * SLOTS
            av_acc = [singles.tile([DK + 1, QB], f32, name=f"av_acc{j}")
                      for j in range(SLOTS)]

            def emit_proj(c0, W=CHUNK, xsrc=None, kbase=0):
                # project x cols [c0, c0+W) (from the packed stream by
                # default, or from xq for the own blocks) into kT/vt at
                # block base kbase + c0//QB
                xs = xsrc if xsrc is not None else xT_sb
                h = W // 2
                cs0 = slice(c0, c0 + h)
                cs1 = slice(c0 + h, c0 + W)
                kv_ps = psB.tile([128, 2, 256], f32, tag="kvps", bufs=2,
                                 name="kv_ps")
                kv0, kv1 = kv_ps[:, 0, 0:h], kv_ps[:, 1, 0:h]
                for d in range(ND):
                    nc.tensor.matmul(kv0, lhsT=wkv_sb[:, d, :],
                                     rhs=xs[:, d, cs0],
                                     start=(d == 0), stop=False,
                                     skip_group_check=True)
                    nc.tensor.matmul(kv1, lhsT=wkv_sb[:, d, :],
                                     rhs=xs[:, d, cs1],
                                     start=False, stop=(d == ND - 1),
                                     skip_group_check=True)
                kc0 = kbase * QB + c0
                nc.vector.tensor_scalar_add(
                    kT_sb[:, kc0:kc0 + W].rearrange("k (h s) -> k h s", h=2),
                    kv_ps[0:DK, :, 0:h], bkv_sb[0:DK, 0:1])
                vT_h = epool.tile([DK, CHUNK], f32, tag="vth", name="vT_h")
                nc.vector.tensor_scalar_add(
                    vT_h[:, 0:W].rearrange("k (h s) -> k h s", h=2),
                    kv_ps[DK:128, :, 0:h], bkv_sb[DK:128, 0:1])
                t_ps = psB.tile([128, 4, QB], f32, tag="scps", bufs=3,
                                name="t_ps")
                for sl in range(W // QB):
                    nc.tensor.transpose(t_ps[:, sl, 0:DK],
                                        vT_h[:, QB * sl:QB * (sl + 1)],
                                        identf_sb[0:DK, 0:DK])
                kb0 = kbase + c0 // QB
                nc.scalar.copy(vt_sb[:, kb0:kb0 + W // QB, 0:DK],
                               t_ps[:, 0:W // QB, 0:DK])

            def emit_attn(kb0, nb, j):
                # attention for q-slot j vs key blocks [kb0, kb0+nb) of the
                # PACKED stream (slot j's packed prefix is 7j+7 blocks;
                # band = packed positions 7j..7j+6, 0/1 mask by rank)
                qc = slice(QB * j, QB * (j + 1))
                sc_ps = psB.tile([128, 4, QB], f32, tag="scps",
                                 bufs=3, name="sc_ps")
                e_sb = epool.tile([128, 4, QB], bf16, tag=f"e{j % 2}",
                                  name="e_sb")
                for sl in range(nb):
                    kb = kb0 + sl
                    nc.tensor.matmul(sc_ps[:, sl, :],
                                     lhsT=kT_sb[:, QB * kb:QB * (kb + 1)],
                                     rhs=qT_sb[:, qc],
                                     start=True, stop=True)
                nc.scalar.activation(e_sb[:, 0:nb, :], sc_ps[:, 0:nb, :],
                                     AF.Exp, scale=0.125)
                band0 = 7 * j
                if kb0 + nb > band0:   # overlaps the band
                    mi = 3 + kb0 - band0   # mask has 3 leading ones blocks
                    nc.vector.tensor_mul(e_sb[:, 0:nb, :], e_sb[:, 0:nb, :],
                                         mask_sb[:, mi:mi + nb, :])
                avp = psB.tile([DK + 1, QB], f32, tag="avp", bufs=2,
                               name="avp")
                for sl in range(nb):
                    kb = kb0 + sl
                    nc.tensor.matmul(avp, lhsT=vt_sb[:, kb, :],
                                     rhs=e_sb[:, sl, :],
                                     start=(sl == 0), stop=(sl == nb - 1),
                                     skip_group_check=True)
                if not av_started[j]:
                    nc.vector.tensor_copy(av_acc[j], avp)
                    av_started[j] = True
                else:
                    nc.vector.tensor_tensor(av_acc[j], av_acc[j], avp,
                                            op=mybir.AluOpType.add)

            def emit_own(j):
                # own key blocks i <= j (tri mask on i == j, static)
                nbo = j + 1
                sc_ps = psB.tile([128, 4, QB], f32, tag="scps",
                                 bufs=3, name="sc_ps")
                e_sb = epool.tile([128, 4, QB], bf16, tag=f"e{j % 2}",
                                  name="e_sb")
                for i in range(nbo):
                    nc.tensor.matmul(
                        sc_ps[:, i, :],
                        lhsT=kT_sb[:, SP + QB * i:SP + QB * (i + 1)],
                        rhs=qT_sb[:, QB * j:QB * (j + 1)],
                        start=True, stop=True)
                nc.scalar.activation(e_sb[:, 0:nbo, :], sc_ps[:, 0:nbo, :],
                                     AF.Exp, scale=0.125)
                nc.vector.tensor_mul(e_sb[:, j, :], e_sb[:, j, :], tri_sb)
                avp = psB.tile([DK + 1, QB], f32, tag="avp", bufs=2,
                               name="avp")
                for i in range(nbo):
                    nc.tensor.matmul(avp, lhsT=vt_sb[:, NPB + i, :],
                                     rhs=e_sb[:, i, :],
                                     start=(i == 0), stop=(i == nbo - 1),
                                     skip_group_check=True)
                if not av_started[j]:
                    nc.vector.tensor_copy(av_acc[j], avp)
                    av_started[j] = True
                else:
                    nc.vector.tensor_tensor(av_acc[j], av_acc[j], avp,
                                            op=mybir.AluOpType.add)

            def emit_epi(j):
                t2 = psB.tile([128, 4, QB], f32, tag="scps", bufs=3,
                              name="t2")
                nc.tensor.transpose(t2[:, 0, 0:DK + 1], av_acc[j],
                                    identf_sb[0:DK + 1, 0:DK + 1])
                rec = epool.tile([128, 1], f32, tag="rec", name="rec")
                nc.vector.reciprocal(rec, t2[:, 0, DK:DK + 1])
                out_sb = epool.tile([128, DK], f32, tag="osb",
                                    name="out_sb")
                nc.vector.tensor_scalar_mul(out_sb, t2[:, 0, 0:DK], rec)
                nc.sync.dma_start(out=out_d[QB * j:QB * (j + 1), :],
                                  in_=out_sb)

            def attn_for_ready(lo, hi):
                # emit attention for every (slot, group) whose packed blocks
                # lie in [lo, hi) -- groups are per-slot, 4-block aligned to
                # the slot's own prefix
                for j in range(SLOTS):
                    pref = 7 * j + 7
                    for g0 in range(0, pref, 4):
                        nb = min(4, pref - g0)
                        if lo < g0 + nb <= hi:
                            emit_attn(g0, nb, j)
                            if g0 + nb == pref:
                                emit_epi(j)

            emit_proj(0, CHUNK)
            # Q^T and own-block K/V from xq
            q_ps = psA.tile([DK, SH], f32, name="q_ps", tag="qps")
            for d in range(ND):
                nc.tensor.matmul(q_ps, lhsT=wq_sb[:, d, :],
                                 rhs=xq_sb[:, d, :],
                                 start=(d == 0), stop=(d == ND - 1))
            nc.scalar.activation(qT_sb, q_ps, AF.Identity,
                                 bias=bq_sb[:, 0:1], scale=1.0)
            emit_proj(0, CHUNK, xsrc=xq_sb, kbase=NPB)
            for j in range(SLOTS):
                emit_own(j)
            done = 4
            attn_for_ready(0, 4)
            for c0 in range(CHUNK, SP - 512, CHUNK):
                emit_proj(c0)
                attn_for_ready(done, done + 4)
                done += 4
            emit_proj(SP - 512, 256)
            attn_for_ready(done, done + 2)
            done += 2
            emit_proj(SP - 256, 256)
            attn_for_ready(done, done + 2)

        for _rep in range(AMP):
            one_pass(_rep)

    nc.finalize()
    return nc


def _in_maps(x, Wq, bq, Wk, bk, Wv, bv):
    bf = ml_dtypes.bfloat16
    xT = np.ascontiguousarray(x.T).astype(bf)                      # [1024, 4096]
    tri = np.triu(np.ones((QB, QB), dtype=np.float32))  # E^T[k,q] valid iff k<=q

    # bf16 constant blobs
    wkvT = np.concatenate([Wk.T, Wv.T], axis=1)                    # [1024, 128]
    wkv_p = wkvT.reshape(ND, 128, 2 * DK).transpose(1, 0, 2).reshape(128, -1)
    wqT = Wq.T                                                     # [1024, 64]
    wq_p = wqT.reshape(ND, 128, DK).transpose(1, 0, 2).reshape(128, -1)
    ident = np.eye(128, dtype=np.float32)
    cbw = np.ascontiguousarray(
        np.concatenate([wkv_p, wq_p, ident], axis=1).astype(bf))
    assert cbw.shape == (128, BFW_COLS)

    # f32 constants
    cf = np.zeros((128, F_COLS), dtype=np.float32)
    cf[:, 0] = np.concatenate([bk, bv])
    cf[0:DK, 1] = bq
    cf[:, 2:2 + 128] = ident

    maps = []
    for c in range(NCORES):
        own = [8 * sl + c for sl in range(SLOTS)]
        rows = np.concatenate([np.arange(QB * b, QB * (b + 1)) for b in own])
        xqT = np.ascontiguousarray(x[rows].T).astype(bf)           # [1024, 512]
        own_cols = rows
        keep = np.setdiff1d(np.arange(S), own_cols)
        xTp = np.ascontiguousarray(xT[:, keep])                    # [1024, 3584]
        # band mask (packed positions of blocks 8j..8j+7 minus own, same for
        # every j): position p_rel < c -> earlier rank block (valid), else 0.
        # 3 leading all-ones pad blocks absorb group windows that start
        # before the band.
        m = np.zeros((10, QB, QB), dtype=np.float32)
        m[0:3] = 1.0
        for p_rel in range(7):
            if p_rel < c:
                m[3 + p_rel] = 1.0
        mask_p = m.transpose(1, 0, 2).reshape(128, -1)
        cbr = np.ascontiguousarray(np.concatenate(
            [mask_p, tri], axis=1).astype(bf))
        assert cbr.shape == (128, BFR_COLS)
        maps.append({"xT": xTp, "xqT": xqT, "cbw": cbw, "cbr": cbr,
                     "cf": cf})
    return maps


def kernel(**inputs):
    global LAST_EXEC_NS
    x = np.asarray(inputs["x"], dtype=np.float32)
    args = [np.asarray(inputs[k], dtype=np.float32)
            for k in ("Wq", "bq", "Wk", "bk", "Wv", "bv")]
    in_maps = _in_maps(x, args[0], args[1], args[2], args[3], args[4], args[5])

    nc = _build_nc()
    from concourse.bass_utils import run_bass_kernel_spmd
    res = run_bass_kernel_spmd(nc, in_maps, core_ids=list(range(NCORES)))
    LAST_EXEC_NS = res.exec_time_ns

    out = np.zeros((S, DK), dtype=np.float32)
    for c in range(NCORES):
        r = res.results[c]["out"]
        for sl in range(SLOTS):
            b = 8 * sl + c
            out[QB * b:QB * (b + 1)] = r[QB * sl:QB * (sl + 1)]
    return out
